# revision 28
# baseline (speedup 1.0000x reference)
"""Trainium2 Bass kernel: DeepseekV4 CSA Compressor.

Math (per batch b):
  kv = hidden @ w_kv, gate = hidden @ w_gate          [S, 256]
  windows w = 0..S/32-1: tokens [w*32-32, w*32+32)  (prev block -> lo
  channels, current block -> hi channels; window 0 prev = 0 kv / -inf gate)
  pooled[w] = sum_j softmax_j(win_g + pos_bias)[j, d] * win_kv[j, d]
  RoPE on trailing 64 dims at position w*32.

Sharding: 8 cores = (4 batches) x (2 sequence halves).  Each core gets its
4096-token chunk transposed on host ([H, 4128] with a 32-token halo column
block in front; zeros for the first half).  No collectives.

Matmuls run in bfloat16 (same PE streaming rate as f32r -- 1 column/cycle
-- but half the HBM traffic and FWL-accelerated weight loads).  End-to-end
bf16 rel err ~2.4e-3 vs the 2e-2 gate.

The position-bias add is folded into the softmax as a multiplicative
exp(bias) pattern applied AFTER the exp (softmax(g+b) weights = e^g * e^b
up to normalization), so the gate PSUM feeds the ACT exp directly and the
first-window -inf fill becomes exp(bias)=0.  Pooling intermediates are
bf16 for 2x DVE throughput; softmax sums/normalization stay fp32.
"""

import ml_dtypes
import numpy as np

HEAD_DIM = 128
ROPE_DIM = 64
RATIO = 32
ROPE_THETA = 10000.0

B, S, H = 4, 8192, 4096
N_CORES = 8
HALF = S // 2                 # tokens per core
NWIN_CORE = HALF // RATIO     # windows per core = 128
GW = 512                      # tokens per matmul/pooling group
WPG = GW // RATIO             # windows per group = 16

_CACHE: dict = {}


def _to_bf16(x: np.ndarray) -> np.ndarray:
    """Convert fp32 to bf16 (round-to-nearest-even)."""
    return np.ascontiguousarray(np.asarray(x, np.float32).astype(ml_dtypes.bfloat16))


def build_program(T_main: int, H_: int, nwin: int):
    """Build the single-core SPMD Bass program.

    T_main: tokens per core (multiple of 2*GW); H_: hidden dim (multiple of
    256); nwin: windows per core (= T_main // RATIO).
    """
    from contextlib import ExitStack

    import concourse.bacc as bacc
    import concourse.mybir as mybir
    import concourse.tile as tile

    f32 = mybir.dt.float32
    bf16 = mybir.dt.bfloat16
    AF = mybir.ActivationFunctionType
    AX = mybir.AxisListType

    d = HEAD_DIM
    r = RATIO
    NG = T_main // GW         # number of groups
    KT = H_ // 128            # k tiles
    C = 4 * d                 # 512 projection channels (kv_lo|kv_hi|g_lo|g_hi)
    NPAIR = NG // 2
    KKT = KT // 2
    PW = 2 * GW + r           # 1056 columns per pair load

    nc = bacc.Bacc("TRN2", target_bir_lowering=False, debug=False,
                   num_devices=N_CORES)
    # Pre-tiled on host in exact consumption order: each [128, 2, PW] block
    # is one fully-contiguous 540KB DMA (sequential HBM streaming).
    hTp = nc.dram_tensor("hTp", [NPAIR, KKT, 128, 2, PW], bf16,
                         kind="ExternalInput").ap()
    Wt = nc.dram_tensor("W", [H_, C], bf16, kind="ExternalInput").ap()
    # exp(position_bias) patterns, [d, 2, GW] (dim1: 0=lo window half,
    # 1=hi half); eb0 zeros the first window's lo half (the -inf gate fill).
    eb_r = nc.dram_tensor("eb_r", [d, 2, GW], bf16, kind="ExternalInput").ap()
    eb_0 = nc.dram_tensor("eb_0", [d, 2, GW], bf16, kind="ExternalInput").ap()
    cos_in = nc.dram_tensor("cos", [nwin, ROPE_DIM // 2], f32,
                            kind="ExternalInput").ap()
    sin_in = nc.dram_tensor("sin", [nwin, ROPE_DIM // 2], f32,
                            kind="ExternalInput").ap()
    ident = nc.dram_tensor("ident", [d, d], f32, kind="ExternalInput").ap()
    out = nc.dram_tensor("out", [nwin, d], f32, kind="ExternalOutput").ap()

    with tile.TileContext(nc) as tc, ExitStack() as ctx:
        wp = ctx.enter_context(tc.tile_pool(name="wp", bufs=1))
        hp = ctx.enter_context(tc.tile_pool(name="hp", bufs=8))
        h0p = ctx.enter_context(tc.tile_pool(name="h0p", bufs=1))
        lp = ctx.enter_context(tc.tile_pool(name="lp", bufs=1))
        pp = ctx.enter_context(tc.tile_pool(name="pp", bufs=2, space="PSUM"))
        sp = ctx.enter_context(tc.tile_pool(name="sp", bufs=2))
        smp = ctx.enter_context(tc.tile_pool(name="smp", bufs=2))
        cp = ctx.enter_context(tc.tile_pool(name="cp", bufs=1))

        # --- PE warmup: the HAM clock gate starts at 1.2 GHz and flips to
        # 2.4 GHz only after ~3.4us of sustained PE activity.  Burn the DMA
        # lead-in (first hidden tile + w0 in flight) on dummy matmuls over
        # memset tiles so the real stream starts at (or near) full clock.
        wz = cp.tile([128, 128], bf16, tag="wz")
        nc.vector.memset(wz[:], 0)
        rz = cp.tile([128, GW], bf16, tag="rz")
        nc.vector.memset(rz[:], 0)
        warm_ps = pp.tile([d, 2, GW], f32, tag="kv2", name="warm")
        for _ in range(4):
            nc.tensor.matmul(warm_ps[:, 0, :], wz[:], rz[:],
                             start=True, stop=True)

        # Stationary weights: one tile per k-tile so the first matmuls only
        # wait on their own chunk (ct: 0=kv_lo 1=kv_hi 2=g_lo 3=g_hi).
        # The 32 tiles stream just-in-time through the two HWDGE rings,
        # interleaved between pair-0 hidden blocks (SWDGE via gpsimd is
        # capped at ~85 GB/s -- too slow for the ~74 GB/s weight demand).
        # w0/w1 are issued here; w_{2kk}/w_{2kk+1} follow block kk below.
        w_sb = []
        w_emitted = set()
        for k in range(KT):
            w_k = wp.tile([128, C], bf16, tag=f"w{k}", name=f"w{k}")
            if k < 2:
                nc.scalar.dma_start(w_k[:], Wt[k * 128:(k + 1) * 128, :])
                w_emitted.add(k)
            w_sb.append(w_k)

        def w_dma(eng, k):
            if k not in w_emitted:
                w_emitted.add(k)
                eng.dma_start(w_sb[k][:], Wt[k * 128:(k + 1) * 128, :])

        ebr = cp.tile([d, 2, GW], bf16, tag="ebr")
        nc.gpsimd.dma_start(ebr[:], eb_r[:])
        eb0 = cp.tile([d, 2, GW], bf16, tag="eb0")
        nc.gpsimd.dma_start(eb0[:], eb_0[:])
        idt = cp.tile([d, d], f32, tag="idt")
        nc.gpsimd.dma_start(idt[:], ident[:])
        cosb = cp.tile([nwin, ROPE_DIM // 2], f32, tag="cosb")
        nc.gpsimd.dma_start(cosb[:], cos_in[:])
        sinb = cp.tile([nwin, ROPE_DIM // 2], f32, tag="sinb")
        nc.gpsimd.dma_start(sinb[:], sin_in[:])

        pooled = cp.tile([d, nwin], f32, tag="pooled")
        # Zero-init so the early finalize can transpose the full tile
        # before the last group's windows are written.
        nc.vector.memset(pooled[:], 0)

        def pooling_group(g, kv2, g2, use_kvc=True):
            # Softmax-gated pooling for one 512-token group.  The kv PSUM
            # banks are copied out to bf16 by ACT first (starts under the
            # tail of the group's own matmul stream) so the banks free at
            # pair end and the next pair's matmuls never stall; the gate
            # banks feed the ACT exp directly and release right after.
            # The very last group skips the copy (no successor to stall)
            # to shorten the end-of-kernel dependency chain.
            # No max-subtraction: gate values are O(6), exp is safe.
            eb = eb0 if g == 0 else ebr
            if use_kvc:
                kvc = sp.tile([d, 2, GW], bf16, tag="kvc", name=f"kvc_{g}")
                nc.scalar.activation(kvc[:], kv2[:], AF.Copy)
                kv_src = kvc
            else:
                kv_src = kv2
            e2 = sp.tile([d, 2, GW], bf16, tag="e2", name=f"e2_{g}")
            nc.scalar.activation(e2[:, 0, :], g2[:, 0, :], AF.Exp)
            nc.scalar.activation(e2[:, 1, :], g2[:, 1, :], AF.Exp)
            ep = sp.tile([d, 2, GW], bf16, tag="ep", name=f"ep_{g}")
            nc.vector.tensor_mul(ep[:], e2[:], eb[:])
            p2 = sp.tile([d, 2, GW], bf16, tag="p2", name=f"p2_{g}")
            nc.vector.tensor_mul(p2[:], ep[:], kv_src[:])
            # Per-window sums over both halves in one segmented reduce:
            # [d, (t w j)] viewed as [d, w, t, j], reduce (t, j).
            sall = smp.tile([d, WPG], f32, tag="sall", name=f"sall_{g}")
            nc.vector.reduce_sum(
                sall[:], ep[:].rearrange("p t (w j) -> p w t j", j=r),
                axis=AX.XY)
            num = smp.tile([d, WPG], f32, tag="num", name=f"num_{g}")
            nc.vector.reduce_sum(
                num[:], p2[:].rearrange("p t (w j) -> p w t j", j=r),
                axis=AX.XY)
            rs = smp.tile([d, WPG], f32, tag="rs", name=f"rs_{g}")
            nc.vector.reciprocal(rs[:], sall[:])
            nc.vector.tensor_mul(pooled[:, g * WPG:(g + 1) * WPG],
                                 num[:], rs[:])

        def group_views(ht_j, side):
            # Matmul rhs views for one group within a pair-block k-slice.
            base = 0 if side == 0 else GW
            lo = ht_j[:, base:base + GW]
            hi = ht_j[:, base + r:base + GW + r]
            return lo, hi

        def emit_mms(k, ht_j, side, kv2, g2, which="both"):
            lo, hi = group_views(ht_j, side)
            st, sp_ = (k == 0), (k == KT - 1)
            views = []
            if which in ("both", "kv"):
                views += [(kv2[:, 0, :], lo, 0), (kv2[:, 1, :], hi, 1)]
            if which in ("both", "g"):
                views += [(g2[:, 0, :], lo, 2), (g2[:, 1, :], hi, 3)]
            for psv, rhs, ct in views:
                nc.tensor.matmul(psv, w_sb[k][:, ct * d:(ct + 1) * d], rhs,
                                 start=st, stop=sp_)

        # Pairs 0..NPAIR-2: both groups interleaved per k-tile (shared DMA
        # and weight tiles).  Last pair: group a's matmuls all run first,
        # then group b's, so a's pooling overlaps b's matmul stream and the
        # end-of-kernel tail is a single group's pooling chain.
        for p in range(NPAIR - 1):
            g0, g1 = 2 * p, 2 * p + 1
            kv2a = pp.tile([d, 2, GW], f32, tag="kv2", name=f"kv2_a{p}")
            g2a = pp.tile([d, 2, GW], f32, tag="g2", name=f"g2_a{p}")
            kv2b = pp.tile([d, 2, GW], f32, tag="kv2", name=f"kv2_b{p}")
            g2b = pp.tile([d, 2, GW], f32, tag="g2", name=f"g2_b{p}")
            prev = None
            for kk in range(KKT):
                if p == 0 and kk == 0:
                    # Split the very first load across both HWDGE rings so
                    # the k=0 matmuls start as early as possible.
                    htA = h0p.tile([128, PW], bf16, tag="ht0A")
                    nc.sync.dma_start(htA[:], hTp[0, 0, :, 0, :])
                    htB = h0p.tile([128, PW], bf16, tag="ht0B")
                    nc.scalar.dma_start(htB[:], hTp[0, 0, :, 1, :])
                    slices = [htA[:], htB[:]]
                elif p == 0 and kk in (1, 2):
                    # Next two blocks also load as half-DMAs so the k=2..5
                    # matmuls aren't gated on a full 540KB transfer during
                    # the bandwidth-crunched pipeline fill.
                    eng = nc.sync if kk == 1 else nc.scalar
                    tA = h0p.tile([128, PW], bf16, tag=f"ht{kk}A",
                                  name=f"ht{kk}A")
                    eng.dma_start(tA[:], hTp[0, kk, :, 0, :])
                    tB = h0p.tile([128, PW], bf16, tag=f"ht{kk}B",
                                  name=f"ht{kk}B")
                    eng.dma_start(tB[:], hTp[0, kk, :, 1, :])
                    w_dma(eng, 2 * kk)
                    w_dma(eng, 2 * kk + 1)
                    slices = [tA[:], tB[:]]
                else:
                    ht2 = hp.tile([128, 2, PW], bf16, tag="ht")
                    if p == 0:
                        # pair 0: odd kk -> sync, even -> scalar, and each
                        # block's weight tiles trail it on the same ring
                        # (arrive ~1.5us before their matmuls need them).
                        dma_eng = nc.sync if kk % 2 == 1 else nc.scalar
                        dma_eng.dma_start(ht2[:], hTp[p, kk])
                        w_dma(dma_eng, 2 * kk)
                        w_dma(dma_eng, 2 * kk + 1)
                    else:
                        dma_eng = nc.sync if kk % 2 == 0 else nc.scalar
                        dma_eng.dma_start(ht2[:], hTp[p, kk])
                    slices = [ht2[:, 0, :], ht2[:, 1, :]]
                # Group b lags one k-step behind group a: a's accumulations
                # stop ~2.5us before pair end, so the ACT release chain
                # (kv copy + exps) frees a's PSUM banks before the next
                # pair's matmuls need them.
                for j in range(2):
                    k = 2 * kk + j
                    emit_mms(k, slices[j], 0, kv2a, g2a)
                    if prev is not None:
                        emit_mms(prev[0], prev[1], 1, kv2b, g2b)
                    prev = (k, slices[j])
            emit_mms(prev[0], prev[1], 1, kv2b, g2b)
            pooling_group(g0, kv2a, g2a)
            pooling_group(g1, kv2b, g2b)

        # Finalize (PE transpose + RoPE + store) a slice of windows.  The
        # first nwin-32 windows finalize under the last group's matmul
        # stream; only the last 32 windows run after the final pooling.
        outsb = cp.tile([nwin, d], f32, tag="outsb")
        nope_w = d - ROPE_DIM
        hw_ = ROPE_DIM // 2

        def finalize(w_lo, w_hi, ptr):
            # Full 128x128 transpose (walrus-proven shape); only rows
            # [w_lo:w_hi] of the result are consumed here.
            nc.tensor.transpose(ptr[:], pooled[:], idt[:])
            ps = ptr[w_lo:w_hi, :]
            ob = outsb[w_lo:w_hi, :]
            nc.vector.tensor_copy(ob[:, 0:nope_w], ps[:, 0:nope_w])
            rp = ps[:, nope_w:d].rearrange("p (a two) -> p a two", two=2)
            re_, ro_ = rp[:, :, 0], rp[:, :, 1]
            opv = ob[:, nope_w:d].rearrange("p (a two) -> p a two", two=2)
            oe_, oo_ = opv[:, :, 0], opv[:, :, 1]
            cs, sn = cosb[w_lo:w_hi, :], sinb[w_lo:w_hi, :]
            t1 = smp.tile([nwin, hw_], f32, tag="t1", name=f"t1_{w_lo}")
            t2 = smp.tile([nwin, hw_], f32, tag="t2", name=f"t2_{w_lo}")
            nc.vector.tensor_mul(t1[w_lo:w_hi, :], re_, cs)
            nc.vector.tensor_mul(t2[w_lo:w_hi, :], ro_, sn)
            nc.vector.tensor_sub(oe_, t1[w_lo:w_hi, :], t2[w_lo:w_hi, :])
            t3 = smp.tile([nwin, hw_], f32, tag="t3", name=f"t3_{w_lo}")
            t4 = smp.tile([nwin, hw_], f32, tag="t4", name=f"t4_{w_lo}")
            nc.vector.tensor_mul(t3[w_lo:w_hi, :], ro_, cs)
            nc.vector.tensor_mul(t4[w_lo:w_hi, :], re_, sn)
            nc.vector.tensor_add(oo_, t3[w_lo:w_hi, :], t4[w_lo:w_hi, :])
            nc.sync.dma_start(out[w_lo:w_hi, :], ob)

        # Last pair: persistent tiles (re-read for group b); group a's
        # matmuls all run first so its pooling (and the early finalize)
        # overlaps group b's matmul stream.
        pl = NPAIR - 1
        w_mid = nwin - RATIO
        lt = []
        for kk in range(KKT):
            ht2 = lp.tile([128, 2, PW], bf16, tag=f"lt{kk}")
            dma_eng = nc.sync if kk % 2 == 0 else nc.scalar
            dma_eng.dma_start(ht2[:], hTp[pl, kk])
            w_dma(dma_eng, 2 * kk)      # no-op unless NPAIR == 1 (sim config)
            w_dma(dma_eng, 2 * kk + 1)
            lt.append(ht2)
        for side in range(2):
            kv2 = pp.tile([d, 2, GW], f32, tag="kv2", name=f"kv2_l{side}")
            g2 = pp.tile([d, 2, GW], f32, tag="g2", name=f"g2_l{side}")
            if side == 0:
                for kk in range(KKT):
                    for j in range(2):
                        emit_mms(2 * kk + j, lt[kk][:, j, :], side, kv2, g2)
            else:
                # Last group: gate matmuls all run first so the softmax
                # chain (exp -> eb mul -> denominator/reciprocal) completes
                # under the kv matmul sweep; after the last matmul only
                # products + numerator reduce + final mul remain.
                for which in ("g", "kv"):
                    for kk in range(KKT):
                        for j in range(2):
                            emit_mms(2 * kk + j, lt[kk][:, j, :], side,
                                     kv2, g2, which)
            pooling_group(2 * pl + side, kv2, g2, use_kvc=(side == 0))
            if side == 0 and w_mid > 0:
                ptrA = pp.tile([nwin, d], f32, tag="kv2", name="ptrA")
                finalize(0, w_mid, ptrA)
        ptrB = pp.tile([nwin, d], f32, tag="g2", name="ptrB")
        finalize(w_mid, nwin, ptrB)

    nc.compile()
    return nc


def _host_inputs(hidden_states, w_kv, w_gate, position_bias,
                 T_main: int, nwin: int, n_cores: int):
    """Build per-core input maps (list of dicts) for the SPMD program."""
    d, r = HEAD_DIM, RATIO
    H_ = hidden_states.shape[2]
    n_total = nwin * n_cores // hidden_states.shape[0]  # windows per batch

    Wfull = np.concatenate([np.asarray(w_kv, np.float32),
                            np.asarray(w_gate, np.float32)], axis=1)
    Wr = _to_bf16(Wfull)

    biasT = np.ascontiguousarray(np.asarray(position_bias, np.float32).T)  # [d, 2r]
    eb = np.exp(biasT.astype(np.float64)).astype(np.float32)
    eb_r = np.empty((d, 2, GW), np.float32)
    eb_r[:, 0, :] = np.tile(eb[:, :r], (1, WPG))
    eb_r[:, 1, :] = np.tile(eb[:, r:], (1, WPG))
    eb_0 = eb_r.copy()
    eb_0[:, 0, :r] = 0.0          # window 0 lo half: -inf gate -> 0 weight
    eb_r = _to_bf16(eb_r)
    eb_0 = _to_bf16(eb_0)

    positions = np.arange(n_total, dtype=np.float32) * r
    inv_freq = 1.0 / (ROPE_THETA ** (
        np.arange(0, ROPE_DIM, 2, dtype=np.float32) / ROPE_DIM))
    freqs = positions[:, None] * inv_freq[None, :]         # [n_total, 32]
    cos = np.cos(freqs).astype(np.float32)
    sin = np.sin(freqs).astype(np.float32)
    ident = np.eye(d, dtype=np.float32)

    hs = np.asarray(hidden_states, np.float32)
    halves_per_batch = n_cores // hs.shape[0]
    NPAIR = T_main // (2 * GW)
    KKT = H_ // 256
    PW = 2 * GW + r
    in_maps = []
    for c in range(n_cores):
        b, hf = c // halves_per_batch, c % halves_per_batch
        start = hf * T_main
        chunk = np.empty((H_, T_main + r), np.float32)
        chunk[:, r:] = hs[b, start:start + T_main].T
        if hf == 0:
            chunk[:, :r] = 0.0
        else:
            chunk[:, :r] = hs[b, start - r:start].T
        chunk = _to_bf16(chunk)
        # Pre-tile into exact DMA consumption order:
        # hTp[pair, kk, p, j, c] = chunk[(2kk+j)*128 + p, pair*1024 + c]
        v = chunk.reshape(KKT, 2, 128, T_main + r)
        hTp = np.ascontiguousarray(
            np.stack([v[:, :, :, p0 * 2 * GW:p0 * 2 * GW + PW]
                      for p0 in range(NPAIR)], axis=0).transpose(0, 1, 3, 2, 4))
        w0 = hf * nwin
        in_maps.append({
            "hTp": hTp,
            "W": Wr,
            "eb_r": eb_r,
            "eb_0": eb_0 if hf == 0 else eb_r,
            "cos": np.ascontiguousarray(cos[w0:w0 + nwin]),
            "sin": np.ascontiguousarray(sin[w0:w0 + nwin]),
            "ident": ident,
        })
    return in_maps


def kernel(hidden_states, w_kv, w_gate, position_bias, _want_profile=False):
    """Full-input, full-output entry point.  Shards over 8 NeuronCores."""
    from concourse.bass_utils import run_bass_kernel_spmd

    hs = np.asarray(hidden_states, np.float32)
    B_, S_, H_ = hs.shape
    n = S_ // RATIO
    if "nc" not in _CACHE:
        _CACHE["nc"] = build_program(HALF, H_, NWIN_CORE)
    nc = _CACHE["nc"]

    in_maps = _host_inputs(hs, w_kv, w_gate, position_bias,
                           HALF, NWIN_CORE, N_CORES)
    kwargs = {}
    if _want_profile:
        import os
        import shutil

        shutil.rmtree("work/prof", ignore_errors=True)
        os.makedirs("work/prof", exist_ok=True)
        kwargs = {"trace": True, "tmpdir": os.path.abspath("work/prof")}
    res = run_bass_kernel_spmd(nc, in_maps, list(range(N_CORES)), **kwargs)

    out = np.empty((B_, n, HEAD_DIM), np.float32)
    halves_per_batch = N_CORES // B_
    for c in range(N_CORES):
        b, hf = c // halves_per_batch, c % halves_per_batch
        out[b, hf * NWIN_CORE:(hf + 1) * NWIN_CORE] = res.results[c]["out"]
    if _want_profile:
        return out, res
    return out
